# revision 1
# baseline (speedup 1.0000x reference)
"""AdaFilter Trainium2 kernel — 8-way data-parallel over (sample, panel) pairs.

The whole network is independent per (sample, panel) image: the encoder is a
grouped conv stack (groups = panels), the dynamic conv is grouped per
sample*panel, the peak-finder CNN is shared 3x3 convs applied per image, and
the final scaling is per panel.  So we shard the 64 images across the 8
NeuronCores (8 images per core), replicate/slice the tiny parameters
per-core, and run one SPMD program (compiled to a NEFF per core) with no
collectives.  The full output is reassembled host-side.
"""

import numpy as np

EPS = 1e-5
N, NPAN, H, W = 2, 32, 185, 388
CORES = 8
PPC = (N * NPAN) // CORES  # images per core = 8

_RUN = None  # cached compiled pmap


def _build():
    import jax
    import jax.numpy as jnp

    def conv2d(x, w, b=None, stride=(1, 1), groups=1):
        y = jax.lax.conv_general_dilated(
            x, w, window_strides=stride, padding="VALID",
            feature_group_count=groups,
            dimension_numbers=("NCHW", "OIHW", "NCHW"))
        if b is not None:
            y = y + b[None, :, None, None]
        return y

    def reflect_pad(x, p):
        return jnp.pad(x, ((0, 0), (0, 0), (p, p), (p, p)), mode="reflect")

    def group_norm(x, gamma, beta, groups):
        n, c, h, w = x.shape
        xg = x.reshape(n, groups, c // groups, h, w)
        mean = xg.mean(axis=(2, 3, 4), keepdims=True)
        var = xg.var(axis=(2, 3, 4), keepdims=True)
        xg = (xg - mean) / jnp.sqrt(var + EPS)
        return (xg.reshape(n, c, h, w) * gamma[None, :, None, None]
                + beta[None, :, None, None])

    def batch_norm_eval(x, gamma, beta):
        scale = gamma / jnp.sqrt(1.0 + EPS)
        return x * scale[None, :, None, None] + beta[None, :, None, None]

    def shard_fn(x, w_e1, b_e1, g_e1, be_e1, w_e2, b_e2, g_e2, be_e2,
                 w_e3, b_e3, g_e3, be_e3, W_lin, b_lin,
                 w_pf1, b_pf1, bn1_g, bn1_b, w_pf2, b_pf2, bn2_g, bn2_b,
                 w_sc, b_sc):
        # x: [1, PPC, H, W] — this core's 8 images, treated as 8 "panels"
        # of a single sample so every grouped op uses groups=PPC.
        relu = jax.nn.relu
        G = PPC
        e = relu(group_norm(conv2d(x, w_e1, b_e1, stride=(4, 8), groups=G),
                            g_e1, be_e1, G))
        e = relu(group_norm(conv2d(e, w_e2, b_e2, stride=(8, 8), groups=G),
                            g_e2, be_e2, G))
        e = relu(group_norm(conv2d(e, w_e3, b_e3, stride=(5, 6), groups=G),
                            g_e3, be_e3, G))
        feat = e.reshape(G, 8)                          # [8, 8]
        wb = feat @ W_lin.T + b_lin                     # [8, 305]
        w1 = wb[:, :144].reshape(G * 16, 1, 3, 3)
        b1 = wb[:, 144:160].reshape(-1)
        w2 = wb[:, 160:304].reshape(G, 16, 3, 3)
        b2 = wb[:, 304:305].reshape(-1)
        f = x  # [1, G, H, W]
        f = conv2d(reflect_pad(f, 1), w1, b1, groups=G)  # [1, G*16, H, W]
        f = conv2d(reflect_pad(f, 1), w2, b2, groups=G)  # [1, G, H, W]
        f = f.reshape(G, 1, H, W)
        y = relu(batch_norm_eval(conv2d(reflect_pad(f, 1), w_pf1, b_pf1),
                                 bn1_g, bn1_b))
        y = relu(batch_norm_eval(conv2d(reflect_pad(y, 1), w_pf2, b_pf2),
                                 bn2_g, bn2_b))
        y = y + f                                        # [G, 1, H, W]
        y = (y * w_sc[:, None, None, None]
             + b_sc[:, None, None, None])
        return y                                         # [G, 1, H, W]

    return jax.pmap(shard_fn, devices=jax.devices()[:CORES])


def kernel(**inputs):
    global _RUN
    if _RUN is None:
        _RUN = _build()

    f32 = lambda a: np.asarray(a, dtype=np.float32)
    x = f32(inputs["x"])                                 # [2, 32, H, W]

    # Global image index g = s*32 + p; core c owns g in [8c, 8c+8).
    xs = x.reshape(N * NPAN, H, W).reshape(CORES, 1, PPC, H, W)

    def enc_slice(a, per):  # rows [per*p0 : per*(p0+PPC)) of an encoder param
        a = f32(a)
        return np.stack([a[per * ((PPC * c) % NPAN):
                           per * ((PPC * c) % NPAN) + per * PPC]
                         for c in range(CORES)])

    def rep(a):  # replicate across cores
        a = f32(a)
        return np.broadcast_to(a[None], (CORES,) + a.shape)

    def pan_slice(a):  # per-panel 32-vector -> this core's 8 entries
        a = f32(a)
        return np.stack([a[(PPC * c) % NPAN:(PPC * c) % NPAN + PPC]
                         for c in range(CORES)])

    args = (
        xs,
        enc_slice(inputs["w_e1"], 2), enc_slice(inputs["b_e1"], 2),
        enc_slice(inputs["g_e1"], 2), enc_slice(inputs["be_e1"], 2),
        enc_slice(inputs["w_e2"], 4), enc_slice(inputs["b_e2"], 4),
        enc_slice(inputs["g_e2"], 4), enc_slice(inputs["be_e2"], 4),
        enc_slice(inputs["w_e3"], 8), enc_slice(inputs["b_e3"], 8),
        enc_slice(inputs["g_e3"], 8), enc_slice(inputs["be_e3"], 8),
        rep(inputs["W_lin"]), rep(inputs["b_lin"]),
        rep(inputs["w_pf1"]), rep(inputs["b_pf1"]),
        rep(inputs["bn1_g"]), rep(inputs["bn1_b"]),
        rep(inputs["w_pf2"]), rep(inputs["b_pf2"]),
        rep(inputs["bn2_g"]), rep(inputs["bn2_b"]),
        pan_slice(inputs["w_sc"]), pan_slice(inputs["b_sc"]),
    )
    out = np.asarray(_RUN(*args))                        # [8, 8, 1, H, W]
    return out.reshape(N * NPAN, 1, H, W).astype(np.float32)
